# revision 1
# baseline (speedup 1.0000x reference)
"""Transformer block kernel for TRN2 (Bass/Tile), one batch element per core.

Computes (per core, x [1024, 768] f32):
    h  = LN(x) (gamma/beta pre-folded into weights on host)
    qk = h @ qkw + qkb ; q = qk[:, :768], k = qk[:, 768:]  (head-major 12x64)
    v  = h @ vw                 (v bias folded into proj bias on host)
    S^T[m,n] = (k_m . q_n) / 8 ;  P = exp(S^T)   (no max subtraction; scores are small)
    oe = [v; 1]^T @ P  -> rows 0..63 = unnormalized o^T, row 64 = softmax denom
    o^T = oe[0:64] / denom
    x1 = x + o @ pw + pb
    h2 = LN2(x1) (folded)
    out = x1 + gelu(h2 @ f1w + f1b) @ f2w + f2b

Layout convention: "feature-major" tensors are [feat_on_partitions, tokens] SBUF
tiles; token-major are [tokens_on_partitions, feat]. LN / residual are
token-major; matmuls contract over partitions so projections run feature-major.
"""

import sys
from contextlib import ExitStack

if "/opt/trn_rl_repo" not in sys.path:
    sys.path.insert(0, "/opt/trn_rl_repo")

import numpy as np

import concourse.bass as bass
import concourse.mybir as mybir
from concourse.masks import make_identity

F32 = mybir.dt.float32
F32R = mybir.dt.float32r
BF16 = mybir.dt.bfloat16
AF = mybir.ActivationFunctionType
ALU = mybir.AluOpType

P = 128
EMB = 768
SEQ = 1024
NH = 12
HD = 64
MLPD = 3072
EC = EMB // P      # 6 embedding chunks
NT = SEQ // P      # 8 token tiles
NC2 = SEQ // 512   # 2 token n-chunks
HC = MLPD // P     # 24 hidden chunks
HP = NH // 2       # 6 head pairs
EPS = 1e-5
SCALE = HD ** -0.5


def r32(ap):
    """Identity; matmul operands are declared float32r at allocation."""
    return ap


def _ln_stats(nc, x_ap, mv, stats, eps_t):
    """bn stats + rstd for one [128, EMB] tile; mv = [mean, rstd]."""
    xg = x_ap.rearrange("p (g d) -> p g d", d=256)
    for g in range(3):
        nc.vector.bn_stats(out=stats[:, g, :], in_=xg[:, g, :])
    nc.vector.bn_aggr(out=mv, in_=stats)
    # rstd = 1/sqrt(var + eps); Sqrt on ACT (one table set), exact recip on DVE
    # ([128,1] is one element per lane - fast)
    nc.scalar.activation(out=mv[:, 1:2], in_=mv[:, 1:2], func=AF.Sqrt, bias=eps_t, scale=1.0)
    nc.vector.reciprocal(out=mv[:, 1:2], in_=mv[:, 1:2])


def _ln_apply(nc, x_ap, h_out, mv):
    nc.vector.tensor_scalar(
        out=h_out,
        in0=x_ap,
        scalar1=mv[:, 0:1],
        scalar2=mv[:, 1:2],
        op0=ALU.subtract,
        op1=ALU.mult,
    )


def _transpose_to_featmajor(nc, tc, pool_ps, pool_sb, src_tok, dstT, t):
    """PE-transpose token-major src_tok [128, EMB] into dstT [:, e, t*128:(t+1)*128]."""
    ident = tc._block_ident
    for group_start, group_n in ((0, 4), (4, 2)):
        ptr = pool_ps.tile([P, 4 * P], BF16, tag="tr", name=f"ptr_t{t}_{group_start}")
        for j in range(group_n):
            e = group_start + j
            nc.tensor.transpose(
                ptr[:, j * P:(j + 1) * P],
                src_tok[:, e * P:(e + 1) * P],
                ident,
            )
        nc.scalar.copy(
            out=dstT[:, group_start:group_start + group_n, t * P:(t + 1) * P],
            in_=ptr[:, :group_n * P].rearrange("p (j q) -> p j q", q=P),
        )


def build_block(tc, outs, ins):
    nc = tc.nc
    x_d = ins["x"]
    qkw_d, qkb_d = ins["qkw"], ins["qkb"]
    vw_d = ins["vw"]
    pw_d, pb_d = ins["pw"], ins["pb"]
    f1w_d, f1b_d = ins["f1w"], ins["f1b"]
    f2w_d, f2b_d = ins["f2w"], ins["f2b"]
    out_d = outs["out"]

    with ExitStack() as ctx:
        consts = ctx.enter_context(tc.tile_pool(name="consts", bufs=1))
        ident = consts.tile([P, P], BF16)
        make_identity(nc, ident)
        tc._block_ident = ident
        eps_t = consts.tile([P, 1], F32)
        nc.vector.memset(eps_t, EPS)
        qkb_sb = consts.tile([P, 2 * EC], F32)
        pb_sb = consts.tile([P, EC], F32)
        f1b_sb = consts.tile([P, HC], F32)
        f2b_sb = consts.tile([P, EC], F32)

        # Persistent SBUF tensors
        glob = ctx.enter_context(tc.tile_pool(name="glob", bufs=1))
        x1 = glob.tile([P, NT, EMB], F32)            # residual stream (starts as x)
        actT = glob.tile([P, EC, SEQ], BF16, tag="actT")  # hT, later h2T reuses slot

        attn_glob = ctx.enter_context(tc.tile_pool(name="attn_glob", bufs=1))
        vext = attn_glob.tile([P, NT, NH, HD + 1], BF16)
        oT = attn_glob.tile([P, EC, SEQ], BF16)      # attention out, feature-major
        vw_sb = attn_glob.tile([P, EC, EMB], BF16)
        pw_sb = attn_glob.tile([P, EC, EMB], BF16)

        work = ctx.enter_context(tc.tile_pool(name="work", bufs=3))
        stat_pool = ctx.enter_context(tc.tile_pool(name="stat", bufs=4))

        # ---- load x into x1 (x1 is BOTH the LN1 input and the residual acc) ----
        x_r = x_d.rearrange("(t p) e -> p t e", p=P)
        for t in range(NT):
            nc.sync.dma_start(out=x1[:, t, :], in_=x_r[:, t, :])

        # ================= Phase A: LN1 + transpose to hT =================
        with tc.tile_pool(name="psA", space="PSUM", bufs=2) as psA:
            hs, mvs = [], []
            for t in range(NT):
                mv = stat_pool.tile([P, 2], F32, tag="mv", bufs=NT, name=f"mv1_{t}")
                stats = stat_pool.tile([P, 3, 6], F32, tag="stats", name=f"st1_{t}")
                _ln_stats(nc, x1[:, t, :], mv, stats, eps_t)
                mvs.append(mv)
            for t in range(NT):
                h_t = work.tile([P, EMB], BF16, tag="h", bufs=NT, name=f"h_{t}")
                _ln_apply(nc, x1[:, t, :], h_t, mvs[t])
                hs.append(h_t)
            for t in range(NT):
                _transpose_to_featmajor(nc, tc, psA, work, hs[t], actT, t)

        # weights / biases (emitted after x+LN so the x DMAs win the queues)
        nc.sync.dma_start(out=vw_sb, in_=vw_d.rearrange("(kc p) o -> p kc o", p=P))
        nc.sync.dma_start(out=qkb_sb, in_=qkb_d.rearrange("(m p) -> p m", p=P))
        nc.sync.dma_start(out=pb_sb, in_=pb_d.rearrange("(m p) -> p m", p=P))
        nc.sync.dma_start(out=f1b_sb, in_=f1b_d.rearrange("(m p) -> p m", p=P))
        nc.sync.dma_start(out=f2b_sb, in_=f2b_d.rearrange("(m p) -> p m", p=P))

        # ================= Phase B: v projection + attention =================
        with tc.tile_pool(name="psB", space="PSUM", bufs=2) as psB:
            # ---- v = h @ vw (token-major), packed into vext with ones column ----
            nc.vector.memset(vext[:, :, :, HD:HD + 1], 1.0)
            for t in range(NT):
                pv = psB.tile([P, 2, 512], F32, tag="mm2", name=f"pv_{t}")
                for half, (c0, cw) in enumerate(((0, 512), (512, 256))):
                    for e in range(EC):
                        nc.tensor.matmul(
                            pv[:, half, :cw],
                            actT[:, e, t * P:(t + 1) * P],
                            vw_sb[:, e, c0:c0 + cw],
                            start=(e == 0),
                            stop=(e == EC - 1),
                        )
                nc.vector.tensor_copy(
                    out=vext[:, t, 0:8, 0:HD],
                    in_=pv[:, 0, :].rearrange("p (h d) -> p h d", d=HD),
                )
                nc.vector.tensor_copy(
                    out=vext[:, t, 8:12, 0:HD],
                    in_=pv[:, 1, 0:256].rearrange("p (h d) -> p h d", d=HD),
                )

            nc.sync.dma_start(out=pw_sb, in_=pw_d.rearrange("(kc p) e -> p kc e", p=P))

            # ---- per head-pair: qk projection (prefetched one pair ahead),
            # ---- then attention for 2 heads
            qkw_r = qkw_d.rearrange("(kc p) o -> p kc o", p=P)

            def emit_qk(hp):
                qkT = {}
                for role, m in (("q", hp), ("k", HP + hp)):
                    wch = work.tile([P, EC, P], BF16, tag="wchunk", name=f"qkw_{role}{hp}")
                    nc.sync.dma_start(out=wch, in_=qkw_r[:, :, m * P:(m + 1) * P])
                    dst = work.tile([P, SEQ], BF16, tag="qkT", bufs=4, name=f"{role}T_{hp}")
                    for n in range(NC2):
                        pqk = psB.tile([P, 512], F32, tag="pqk", bufs=2, name=f"pqk_{role}{hp}n{n}")
                        for e in range(EC):
                            nc.tensor.matmul(
                                pqk,
                                wch[:, e, :],
                                actT[:, e, n * 512:(n + 1) * 512],
                                start=(e == 0),
                                stop=(e == EC - 1),
                            )
                        nc.vector.tensor_scalar_add(
                            out=dst[:, n * 512:(n + 1) * 512],
                            in0=pqk,
                            scalar1=qkb_sb[:, m:m + 1],
                        )
                    qkT[role] = dst
                return qkT

            qkT = emit_qk(0)
            for hp in range(HP):
                cur = qkT
                if hp + 1 < HP:
                    qkT = emit_qk(hp + 1)

                ous = {}
                dpack = stat_pool.tile([4, 512], F32, tag="dpack", bufs=2, name=f"dp_{hp}")
                for sub in range(2):
                    h = 2 * hp + sub
                    doff = sub * HD
                    qs = cur["q"][doff:doff + HD, :]
                    ks = cur["k"][doff:doff + HD, :]
                    po = [
                        psB.tile([P, 512], F32, tag="oacc", bufs=2, name=f"po_h{h}n{n}")
                        for n in range(NC2)
                    ]
                    for mt in range(NT):
                        ps = psB.tile([P, 2, 512], F32, tag="mm2", name=f"ps_h{h}m{mt}")
                        for n in range(NC2):
                            nc.tensor.matmul(
                                ps[:, n, :],
                                ks[:, mt * P:(mt + 1) * P],
                                qs[:, n * 512:(n + 1) * 512],
                                start=True,
                                stop=True,
                            )
                        pp = work.tile([P, 2, 512], BF16, tag="ppair", bufs=4, name=f"pp_h{h}m{mt}")
                        nc.scalar.activation(out=pp, in_=ps, func=AF.Exp, scale=SCALE)
                        for n in range(NC2):
                            nc.tensor.matmul(
                                po[n][0:HD + 1, :],
                                vext[:, mt, h, :],
                                pp[:, n, :],
                                start=(mt == 0),
                                stop=(mt == NT - 1),
                            )
                    for n in range(NC2):
                        # copy out of PSUM right away so the accumulator bank
                        # recycles without waiting on the normalize chain
                        ou = work.tile([HD + 1, 512], F32, tag="ou", bufs=8, name=f"ou_h{h}n{n}")
                        nc.vector.tensor_copy(out=ou, in_=po[n][0:HD + 1, :])
                        idx = sub * NC2 + n
                        nc.sync.dma_start(out=dpack[idx:idx + 1, :], in_=ou[HD:HD + 1, :])
                        ous[idx] = ou
                # one exact reciprocal for the whole head-pair's denominators
                rpack = stat_pool.tile([4, 512], F32, tag="rpack", bufs=2, name=f"rp_{hp}")
                nc.vector.reciprocal(out=rpack, in_=dpack)
                for sub in range(2):
                    doff = sub * HD
                    for n in range(NC2):
                        idx = sub * NC2 + n
                        rtmp = stat_pool.tile([1, 512], F32, tag="rtmp", bufs=4, name=f"rt_{hp}i{idx}")
                        nc.sync.dma_start(out=rtmp, in_=rpack[idx:idx + 1, :])
                        rb = work.tile([HD, 512], F32, tag="rb", bufs=4, name=f"rb_{hp}i{idx}")
                        nc.gpsimd.partition_broadcast(rb, rtmp)
                        nc.vector.tensor_tensor(
                            out=oT[doff:doff + HD, hp, n * 512:(n + 1) * 512],
                            in0=ous[idx][0:HD, :],
                            in1=rb,
                            op=ALU.mult,
                        )

        # ====== Phase C: proj + residual + LN2, one 512-token chunk at a time ======
        with tc.tile_pool(name="psC", space="PSUM", bufs=2) as psC:
            for n in range(NC2):
                for me in range(EC):
                    ppr = psC.tile([P, 512], F32, tag="mm", name=f"ppr_{me}_{n}")
                    for kc in range(EC):
                        nc.tensor.matmul(
                            ppr,
                            pw_sb[:, kc, me * P:(me + 1) * P],
                            oT[:, kc, n * 512:(n + 1) * 512],
                            start=(kc == 0),
                            stop=(kc == EC - 1),
                        )
                    prn = work.tile([P, 512], BF16, tag="prn", name=f"prn_{me}_{n}")
                    nc.scalar.activation(
                        out=prn, in_=ppr, func=AF.Identity, bias=pb_sb[:, me:me + 1]
                    )
                    ptr = psC.tile([P, 4, P], BF16, tag="tr", name=f"trp_{me}_{n}")
                    for j in range(4):
                        nc.tensor.transpose(ptr[:, j, :], prn[:, j * P:(j + 1) * P], ident)
                    nc.vector.tensor_tensor(
                        out=x1[:, 4 * n:4 * n + 4, me * P:(me + 1) * P],
                        in0=x1[:, 4 * n:4 * n + 4, me * P:(me + 1) * P],
                        in1=ptr,
                        op=ALU.add,
                    )
                hs2, mvs2 = [], []
                for j in range(4):
                    t = 4 * n + j
                    mv = stat_pool.tile([P, 2], F32, tag="mv", bufs=NT, name=f"mv2_{t}")
                    stats = stat_pool.tile([P, 3, 6], F32, tag="stats", name=f"st2_{t}")
                    _ln_stats(nc, x1[:, t, :], mv, stats, eps_t)
                    mvs2.append(mv)
                for j in range(4):
                    t = 4 * n + j
                    h_t = work.tile([P, EMB], BF16, tag="h", bufs=NT, name=f"h2_{t}")
                    _ln_apply(nc, x1[:, t, :], h_t, mvs2[j])
                    hs2.append(h_t)
                for j in range(4):
                    _transpose_to_featmajor(nc, tc, psC, work, hs2[j], actT, 4 * n + j)
        h2T = actT

        # ================= Phase F: MLP + residual + output =================
        out_r = out_d.rearrange("(t p) e -> p t e", p=P)
        f1w_r = f1w_d.rearrange("(kc p) o -> p kc o", p=P)
        f2w_r = f2w_d.rearrange("(hc p) e -> p hc e", p=P)
        with tc.tile_pool(name="psF", space="PSUM", bufs=1) as psF:
            for n in range(NC2):
                acc = [
                    psF.tile([P, 2, 512], F32, tag=f"acc{i}", bufs=1, name=f"acc_{n}_{i}")
                    for i in range(3)
                ]

                def acc_sl(e):
                    return acc[e // 2][:, e % 2, :]

                for hc in range(HC):
                    w1 = work.tile([P, EC, P], BF16, tag="wchunk", name=f"f1w_{n}_{hc}")
                    nc.sync.dma_start(out=w1, in_=f1w_r[:, :, hc * P:(hc + 1) * P])
                    w2 = work.tile([P, EMB], BF16, tag="w2chunk", name=f"f2w_{n}_{hc}")
                    nc.sync.dma_start(out=w2, in_=f2w_r[:, hc, :])
                    pf1 = psF.tile([P, 512], F32, tag="f1", bufs=2, name=f"pf1_{n}_{hc}")
                    for e in range(EC):
                        nc.tensor.matmul(
                            pf1,
                            w1[:, e, :],
                            h2T[:, e, n * 512:(n + 1) * 512],
                            start=(e == 0),
                            stop=(e == EC - 1),
                        )
                    a = work.tile([P, 512], BF16, tag="act", name=f"act_{n}_{hc}")
                    nc.scalar.activation(
                        out=a, in_=pf1, func=AF.Gelu, bias=f1b_sb[:, hc:hc + 1]
                    )
                    for e in range(EC):
                        nc.tensor.matmul(
                            acc_sl(e),
                            w2[:, e * P:(e + 1) * P],
                            a,
                            start=(hc == 0),
                            stop=(hc == HC - 1),
                        )
                for e in range(EC):
                    fr = work.tile([P, 512], BF16, tag="prn", name=f"fr_{n}_{e}")
                    nc.scalar.activation(
                        out=fr, in_=acc_sl(e), func=AF.Identity, bias=f2b_sb[:, e:e + 1]
                    )
                    ptr = psF.tile([P, 4, P], BF16, tag="f1", bufs=2, name=f"trf_{n}_{e}")
                    for j in range(4):
                        nc.tensor.transpose(ptr[:, j, :], fr[:, j * P:(j + 1) * P], ident)
                    nc.vector.tensor_tensor(
                        out=x1[:, 4 * n:4 * n + 4, e * P:(e + 1) * P],
                        in0=x1[:, 4 * n:4 * n + 4, e * P:(e + 1) * P],
                        in1=ptr,
                        op=ALU.add,
                    )
                for j in range(4):
                    t = 4 * n + j
                    nc.sync.dma_start(out=out_r[:, t, :], in_=x1[:, t, :])


def fold_inputs(inputs):
    """Fold LN gamma/beta and v-bias into downstream weights (exact math).

    Returns the dict of effective tensors the kernel consumes.
    """
    f = {k: np.asarray(v, dtype=np.float32) for k, v in inputs.items()}
    qkw = f["ln1_g"][:, None] * f["qk_w"]
    qkb = f["ln1_b"] @ f["qk_w"]
    vw = f["ln1_g"][:, None] * f["v_w"]
    vb = f["ln1_b"] @ f["v_w"]
    # softmax rows sum to 1 => o = attn @ (v + 1 vb^T) = attn@v + vb
    pb = f["proj_b"] + vb @ f["proj_w"]
    f1w = f["ln2_g"][:, None] * f["fc1_w"]
    f1b = f["fc1_b"] + f["ln2_b"] @ f["fc1_w"]
    import ml_dtypes

    bf16 = ml_dtypes.bfloat16
    return {
        "qkw": np.ascontiguousarray(qkw.astype(bf16)),
        "qkb": np.ascontiguousarray(qkb),
        "vw": np.ascontiguousarray(vw.astype(bf16)),
        "pw": np.ascontiguousarray(f["proj_w"].astype(bf16)),
        "pb": np.ascontiguousarray(pb),
        "f1w": np.ascontiguousarray(f1w.astype(bf16)),
        "f1b": np.ascontiguousarray(f1b),
        "f2w": np.ascontiguousarray(f["fc2_w"].astype(bf16)),
        "f2b": np.ascontiguousarray(f["fc2_b"]),
    }


_INPUT_SHAPES = {
    "x": (SEQ, EMB),
    "qkw": (EMB, 2 * EMB),
    "qkb": (2 * EMB,),
    "vw": (EMB, EMB),
    "pw": (EMB, EMB),
    "pb": (EMB,),
    "f1w": (EMB, MLPD),
    "f1b": (MLPD,),
    "f2w": (MLPD, EMB),
    "f2b": (EMB,),
}

_N_CORES = 8
_compiled = {}


def _build_nc(num_devices=_N_CORES):
    import concourse.tile as tile
    from concourse import bacc

    nc = bacc.Bacc(
        "TRN2", target_bir_lowering=False, debug=False, num_devices=num_devices
    )
    _BF16_INPUTS = {"qkw", "vw", "pw", "f1w", "f2w"}
    ins = {
        name: nc.dram_tensor(
            name, list(shape), BF16 if name in _BF16_INPUTS else F32,
            kind="ExternalInput",
        ).ap()
        for name, shape in _INPUT_SHAPES.items()
    }
    out = nc.dram_tensor("out", [SEQ, EMB], F32, kind="ExternalOutput").ap()
    with tile.TileContext(nc) as tc:
        build_block(tc, {"out": out}, ins)
    nc.compile()
    return nc


def kernel(**inputs):
    """Full-input entry point: x [8, 1024, 768] + weights -> [8, 1024, 768]."""
    from concourse.bass_utils import run_bass_kernel_spmd

    if "nc" not in _compiled:
        _compiled["nc"] = _build_nc()
    nc = _compiled["nc"]

    x = np.asarray(inputs["x"], dtype=np.float32)
    folded = fold_inputs({k: v for k, v in inputs.items() if k != "x"})
    in_maps = [
        {"x": np.ascontiguousarray(x[c]), **folded} for c in range(_N_CORES)
    ]
    res = run_bass_kernel_spmd(nc, in_maps, core_ids=list(range(_N_CORES)))
    return np.stack([res.results[c]["out"] for c in range(_N_CORES)]).astype(
        np.float32
    )



# revision 9
# speedup vs baseline: 1.1844x; 1.1844x over previous
"""Transformer block kernel for TRN2 (Bass/Tile), one batch element per core.

fp8-e4m3 DoubleRow edition. All projections (qk, v, proj, fc1, fc2) run as
fp8 DoubleRow matmuls (256-deep contraction per pass, 2x fp8 throughput);
attention scores run fp8 normal-mode with two heads row-packed onto the PE
array (tile_position via base_partition 0/64); AV runs fp8 DoubleRow over
key-tile pairs with the softmax denominator riding as a ones column.

Numerics: per-tensor pow2 scales keep everything in e4m3 range
(weights x2^12/13, LN outputs x32 folded into rstd, attention o x256 folded
into the normalize, exp biased by ln8). Descales fold into existing DVE
tensor_scalar / scalar_tensor_tensor ops and ACT's input affine, so they
cost nothing. End-to-end L2 err ~1e-2 (budget 2e-2), CPU-simulated.

ACT table sets: LN rstd uses Ln+Exp (same set as attention's exp), and all
gelu is deferred to one block per chunk, so the ACT tables load only twice.

Pipeline (per core): LN1+transpose -> v -> qk(hp0) -> attn chunk c0
(queries 0..511, qk prefetch as PE filler) -> attn c1 with proj/LN2/
transpose/fc1 of c0 interleaved as PE fillers under the ACT-bound exp
stream -> tail: proj/LN2(c1), gelu+fc2(c0), fc1(c1), gelu+fc2(c1).
"""

import math
import sys
from contextlib import ExitStack

if "/opt/trn_rl_repo" not in sys.path:
    sys.path.insert(0, "/opt/trn_rl_repo")

import numpy as np

import concourse.bass as bass
import concourse.mybir as mybir
from concourse.masks import make_identity

F32 = mybir.dt.float32
BF16 = mybir.dt.bfloat16
FP8 = mybir.dt.float8e4
AF = mybir.ActivationFunctionType
ALU = mybir.AluOpType
DR = mybir.MatmulPerfMode.DoubleRow

P = 128
EMB = 768
SEQ = 1024
NH = 12
HD = 64
MLPD = 3072
EC = EMB // P       # 6 embedding chunks
NT = SEQ // P       # 8 token tiles
HC = MLPD // P      # 24 hidden chunks
HP = NH // 2        # 6 head pairs
KP = EMB // 256     # 3 k-pairs (DoubleRow) for 768 contraction
KP2 = MLPD // 256   # 12 k-pairs for 3072 contraction
EPS = 1e-5
SCALE = HD ** -0.5  # applied inside exp's input affine

# pow2 scale exponents (weights scaled on host, activations on device)
WEXP = 12           # qkw/vw/pw/f1w weight scale 2^12 (|w|max ~0.036 -> ~148)
WEXP2 = 13          # f2w scale 2^13 (|w|max ~0.018 -> ~148)
HEXP = 5            # LN outputs scaled x32 (folded into rstd)
OEXP = 8            # attention o scaled x256 (folded into normalize)
PBIAS = math.log(8.0)  # exp(s/8 + ln8): p in (0, ~128], cancels in softmax
DQK = 2.0 ** -(WEXP + HEXP)
DV = 2.0 ** -(WEXP + HEXP)
DPR = 2.0 ** -(WEXP + OEXP)
DF1 = 2.0 ** -(WEXP + HEXP)
DF2 = 2.0 ** -WEXP2


def _ln_stats(nc, x_ap, mv, stats, eps_t):
    """bn stats; mv[:,0]=mean, mv[:,1]=32/sqrt(var+eps) via Ln+Exp (one
    ACT table set with attention's exp; no Sqrt-set load)."""
    xg = x_ap.rearrange("p (g d) -> p g d", d=256)
    for g in range(3):
        nc.vector.bn_stats(out=stats[:, g, :], in_=xg[:, g, :])
    nc.vector.bn_aggr(out=mv, in_=stats)
    # ln((var+eps)/1024) then exp(-0.5 * that) = 32/sqrt(var+eps)
    nc.scalar.activation(
        out=mv[:, 1:2], in_=mv[:, 1:2], func=AF.Ln,
        bias=eps_t, scale=1.0 / 1024.0,
    )
    nc.scalar.activation(out=mv[:, 1:2], in_=mv[:, 1:2], func=AF.Exp, scale=-0.5)


def _ln_apply(nc, x_ap, h_out, mv):
    nc.vector.tensor_scalar(
        out=h_out,
        in0=x_ap,
        scalar1=mv[:, 0:1],
        scalar2=mv[:, 1:2],
        op0=ALU.subtract,
        op1=ALU.mult,
    )


def _transpose_fm(nc, pool, tag, src_tok, dstT, t, ident, copy_engine):
    """PE-transpose token-major src [128, EMB] into dstT [:, e, t*128:...]."""
    for g0, gn in ((0, 4), (4, 2)):
        ptr = pool.tile([P, 4, P], BF16, tag=tag, name=f"ptr_{dstT.tensor.name}_t{t}_{g0}")
        for jj in range(gn):
            nc.tensor.transpose(ptr[:, jj, :], src_tok[:, (g0 + jj) * P:(g0 + jj + 1) * P], ident)
        copy_engine(out=dstT[:, g0:g0 + gn, t * P:(t + 1) * P], in_=ptr[:, 0:gn, :])


def build_block(tc, outs, ins):
    nc = tc.nc
    x_d = ins["x"]
    out_d = outs["out"]
    out_r = out_d.rearrange("(t p) e -> p t e", p=P)

    with ExitStack() as ctx:
        consts = ctx.enter_context(tc.tile_pool(name="consts", bufs=1))
        ident = consts.tile([P, P], BF16)
        make_identity(nc, ident)
        qkb_sb = consts.tile([P, 2 * EC], F32)
        f1b_sb = consts.tile([P, HC], F32)
        eps_t = consts.tile([P, 1], F32)
        nc.vector.memset(eps_t, EPS / 1024.0)
        pbias_t = consts.tile([P, 1], F32)
        nc.vector.memset(pbias_t, PBIAS)

        glob = ctx.enter_context(tc.tile_pool(name="glob", bufs=1))
        x1 = glob.tile([P, NT, EMB], F32)          # residual stream, token-major
        hT = glob.tile([P, EC, SEQ], FP8)          # LN1 out x32, feature-major
        h2T = glob.tile([P, EC, SEQ], FP8)         # LN2 out x32, feature-major
        qkT = glob.tile([P, 2 * EC, SEQ], FP8)     # q (m 0..5) / k (m 6..11)
        vext = glob.tile([P, NT, NH, 68], FP8)     # v + ones col (64) + pad
        oT = glob.tile([P, EC, SEQ], FP8)          # attention out x256
        a_raw = glob.tile([P, HC, 512], BF16)      # pre-gelu (descaled+bias)
        a_fm = glob.tile([P, HC, 512], FP8)        # post-gelu
        qkw_sb = glob.tile([P, KP, 2, 2 * EMB], FP8)
        vw_sb = glob.tile([P, KP, 2, EMB], FP8)
        pw_sb = glob.tile([P, KP, 2, EMB], FP8)
        f1w_sb = glob.tile([P, KP, 2, MLPD], FP8)
        f2w_sb = glob.tile([P, KP2, 2, EMB], FP8)
        pbb_sb = glob.tile([P, EMB], F32)
        f2bb_sb = glob.tile([P, EMB], F32)

        work = ctx.enter_context(tc.tile_pool(name="work", bufs=3))
        stat = ctx.enter_context(tc.tile_pool(name="stat", bufs=4))

        # ---- load x, LN1, transpose to hT; then x1 += proj-bias ----
        x_r = x_d.rearrange("(t p) e -> p t e", p=P)
        for t in range(NT):
            nc.sync.dma_start(out=x1[:, t, :], in_=x_r[:, t, :])

        with tc.tile_pool(name="psA", space="PSUM", bufs=2) as psA:
            mvs = []
            for t in range(NT):
                mv = stat.tile([P, 2], F32, tag="mv", bufs=NT, name=f"mv1_{t}")
                stats = stat.tile([P, 3, 6], F32, tag="stats", name=f"st1_{t}")
                _ln_stats(nc, x1[:, t, :], mv, stats, eps_t)
                mvs.append(mv)
            for t in range(NT):
                h_t = work.tile([P, EMB], BF16, tag="h", bufs=4, name=f"h1_{t}")
                _ln_apply(nc, x1[:, t, :], h_t, mvs[t])
                _transpose_fm(nc, psA, "trA", h_t, hT, t, ident, nc.scalar.copy)

        # weights / biases (emitted after x+LN so the x DMAs win the queues)
        nc.sync.dma_start(out=qkw_sb, in_=ins["qkw"])
        nc.sync.dma_start(out=vw_sb, in_=ins["vw"])
        nc.sync.dma_start(out=qkb_sb, in_=ins["qkb"].rearrange("(m p) -> p m", p=P))
        nc.sync.dma_start(out=pbb_sb, in_=ins["pbb"])
        nc.sync.dma_start(out=pw_sb, in_=ins["pw"])
        nc.sync.dma_start(out=f1w_sb, in_=ins["f1w"])
        nc.sync.dma_start(out=f1b_sb, in_=ins["f1b"].rearrange("(m p) -> p m", p=P))
        nc.sync.dma_start(out=f2w_sb, in_=ins["f2w"])
        nc.sync.dma_start(out=f2bb_sb, in_=ins["f2bb"])

        for t in range(NT):
            nc.vector.tensor_tensor(
                out=x1[:, t, :], in0=x1[:, t, :], in1=pbb_sb, op=ALU.add
            )

        with tc.tile_pool(name="psS", space="PSUM", bufs=2) as psS, \
             tc.tile_pool(name="psO", space="PSUM", bufs=2) as psO, \
             tc.tile_pool(name="psM", space="PSUM", bufs=2) as psM:

            # ---- v projection (flipped: stationary=hT, moving=vw) ----
            nc.vector.memset(vext[:, :, :, 64:65], 1.0)
            nc.vector.memset(vext[:, :, :, 65:68], 0.0)
            for t in range(NT):
                pv = psM.tile([P, 512], F32, tag="mlp", name=f"pv_{t}")
                pv2 = psM.tile([P, 512], F32, tag="mlp", name=f"pv2_{t}")
                for j in range(KP):
                    nc.tensor.matmul(
                        pv,
                        hT[:, 2 * j:2 * j + 2, t * P:(t + 1) * P],
                        vw_sb[:, j, :, 0:512],
                        start=(j == 0), stop=(j == KP - 1), perf_mode=DR,
                    )
                for j in range(KP):
                    nc.tensor.matmul(
                        pv2[:, 0:256],
                        hT[:, 2 * j:2 * j + 2, t * P:(t + 1) * P],
                        vw_sb[:, j, :, 512:768],
                        start=(j == 0), stop=(j == KP - 1), perf_mode=DR,
                    )
                nc.vector.tensor_scalar_mul(
                    out=vext[:, t, 0:8, 0:64],
                    in0=pv.rearrange("p (h d) -> p h d", d=HD),
                    scalar1=DV,
                )
                nc.vector.tensor_scalar_mul(
                    out=vext[:, t, 8:12, 0:64],
                    in0=pv2[:, 0:256].rearrange("p (h d) -> p h d", d=HD),
                    scalar1=DV,
                )

            # ---- qk projection for one head pair (2 m-tiles) ----
            def emit_qk(hp):
                for m in (hp, HP + hp):
                    for n2 in range(2):
                        pqk = psM.tile([P, 512], F32, tag="mlp", name=f"pqk_{m}_{n2}")
                        for j in range(KP):
                            nc.tensor.matmul(
                                pqk,
                                qkw_sb[:, j, :, m * P:(m + 1) * P],
                                hT[:, 2 * j:2 * j + 2, n2 * 512:(n2 + 1) * 512],
                                start=(j == 0), stop=(j == KP - 1), perf_mode=DR,
                            )
                        nc.vector.tensor_scalar(
                            out=qkT[:, m, n2 * 512:(n2 + 1) * 512],
                            in0=pqk,
                            scalar1=DQK,
                            scalar2=qkb_sb[:, m:m + 1],
                            op0=ALU.mult,
                            op1=ALU.add,
                        )

            def emit_some(fillers, fi, budget):
                n = 0
                while fi < len(fillers) and n < budget:
                    fillers[fi]()
                    fi += 1
                    n += 1
                return fi

            # ---- attention for one 512-query chunk ----
            def attn_chunk(c, fillers):
                fi = 0
                for hp in range(HP):
                    if c == 0 and hp + 1 < HP:
                        emit_qk(hp + 1)
                    po = [
                        psO.tile([68, 512], F32, tag="po", name=f"po_c{c}h{2 * hp + s}")
                        for s in range(2)
                    ]
                    for i in range(4):  # key-tile pairs
                        pst = [
                            psS.tile([P, 2, 512], F32, tag="ps",
                                     name=f"ps_c{c}hp{hp}i{i}s{s}")
                            for s in range(2)
                        ]
                        # interleave heads A/B: adjacent MMs hit different
                        # PE row strips (base_partition 0 / 64) and overlap
                        for kk in range(2):
                            kt = 2 * i + kk
                            for sub in range(2):
                                doff = sub * HD
                                nc.tensor.matmul(
                                    pst[sub][:, kk, :],
                                    qkT[doff:doff + HD, HP + hp, kt * P:(kt + 1) * P],
                                    qkT[doff:doff + HD, hp, c * 512:(c + 1) * 512],
                                    start=True, stop=True,
                                )
                        pps = []
                        for sub in range(2):
                            ppt = work.tile([P, 2, 512], FP8, tag="pp", bufs=4,
                                            name=f"pp_c{c}hp{hp}i{i}s{sub}")
                            nc.scalar.activation(
                                out=ppt, in_=pst[sub], func=AF.Exp,
                                bias=pbias_t, scale=SCALE,
                            )
                            pps.append(ppt)
                        fi = emit_some(fillers, fi, 1)
                        for sub in range(2):
                            h = 2 * hp + sub
                            nc.tensor.matmul(
                                po[sub],
                                vext[:, 2 * i:2 * i + 2, h, :],
                                pps[sub],
                                start=(i == 0), stop=(i == 3), perf_mode=DR,
                            )
                    # normalize: o = num/den (x256 for fp8 range)
                    dpack = stat.tile([2, 512], F32, tag="dpack", bufs=2,
                                      name=f"dp_c{c}hp{hp}")
                    ous = []
                    for sub in range(2):
                        ou = work.tile([65, 512], F32, tag="ou", bufs=4,
                                       name=f"ou_c{c}hp{hp}s{sub}")
                        nc.vector.tensor_copy(out=ou, in_=po[sub][0:65, :])
                        nc.sync.dma_start(out=dpack[sub:sub + 1, :], in_=ou[64:65, :])
                        ous.append(ou)
                    rpack = stat.tile([2, 512], F32, tag="rpack", bufs=2,
                                      name=f"rp_c{c}hp{hp}")
                    nc.vector.reciprocal(out=rpack, in_=dpack)
                    for sub in range(2):
                        rtmp = stat.tile([1, 512], F32, tag="rtmp", bufs=4,
                                         name=f"rt_c{c}hp{hp}s{sub}")
                        nc.sync.dma_start(out=rtmp, in_=rpack[sub:sub + 1, :])
                        rb = work.tile([HD, 512], F32, tag="rb", bufs=4,
                                       name=f"rb_c{c}hp{hp}s{sub}")
                        nc.gpsimd.partition_broadcast(rb, rtmp)
                        nc.vector.scalar_tensor_tensor(
                            out=oT[sub * HD:sub * HD + HD, hp, c * 512:(c + 1) * 512],
                            in0=ous[sub][0:HD, :],
                            scalar=float(2 ** OEXP),
                            in1=rb,
                            op0=ALU.mult, op1=ALU.mult,
                        )
                    fi = emit_some(fillers, fi, 1)
                emit_some(fillers, fi, len(fillers))

            # ---- MLP pieces for chunk c (emitted as fillers / tail) ----
            def proj_ln2_tr(c):
                fs = []
                for tl in range(4):
                    t = 4 * c + tl

                    def proj_t(t=t):
                        ppr = psM.tile([P, 512], F32, tag="mlp", name=f"ppr_{t}")
                        ppr2 = psM.tile([P, 512], F32, tag="mlp", name=f"ppr2_{t}")
                        for j in range(KP):
                            nc.tensor.matmul(
                                ppr,
                                oT[:, 2 * j:2 * j + 2, t * P:(t + 1) * P],
                                pw_sb[:, j, :, 0:512],
                                start=(j == 0), stop=(j == KP - 1), perf_mode=DR,
                            )
                        for j in range(KP):
                            nc.tensor.matmul(
                                ppr2[:, 0:256],
                                oT[:, 2 * j:2 * j + 2, t * P:(t + 1) * P],
                                pw_sb[:, j, :, 512:768],
                                start=(j == 0), stop=(j == KP - 1), perf_mode=DR,
                            )
                        nc.vector.scalar_tensor_tensor(
                            out=x1[:, t, 0:512], in0=ppr, scalar=DPR,
                            in1=x1[:, t, 0:512], op0=ALU.mult, op1=ALU.add,
                        )
                        nc.vector.scalar_tensor_tensor(
                            out=x1[:, t, 512:768], in0=ppr2[:, 0:256], scalar=DPR,
                            in1=x1[:, t, 512:768], op0=ALU.mult, op1=ALU.add,
                        )
                        if "__dbg" in outs:
                            nc.sync.dma_start(
                                out=outs["__dbg"]["x1p"].rearrange(
                                    "(t p) e -> p t e", p=P)[:, t, :],
                                in_=x1[:, t, :],
                            )

                    def ln2_t(t=t):
                        mv = stat.tile([P, 2], F32, tag="mv", bufs=NT, name=f"mv2_{t}")
                        stats = stat.tile([P, 3, 6], F32, tag="stats", name=f"st2_{t}")
                        _ln_stats(nc, x1[:, t, :], mv, stats, eps_t)
                        h_t = work.tile([P, EMB], BF16, tag="h", bufs=4, name=f"h2_{t}")
                        _ln_apply(nc, x1[:, t, :], h_t, mv)
                        # f2 bias joins the stream only after LN2 consumed x1
                        nc.vector.tensor_tensor(
                            out=x1[:, t, :], in0=x1[:, t, :], in1=f2bb_sb, op=ALU.add
                        )
                        _transpose_fm(nc, psS, "ps", h_t, h2T, t, ident,
                                      nc.vector.tensor_copy)

                    fs.append(proj_t)
                    fs.append(ln2_t)
                return fs

            def fc1_units(c):
                fs = []
                for hc in range(HC):
                    def fc1_hc(hc=hc, c=c):
                        pf1 = psM.tile([P, 512], F32, tag="mlp", name=f"pf1_c{c}_{hc}")
                        for j in range(KP):
                            nc.tensor.matmul(
                                pf1,
                                f1w_sb[:, j, :, hc * P:(hc + 1) * P],
                                h2T[:, 2 * j:2 * j + 2, c * 512:(c + 1) * 512],
                                start=(j == 0), stop=(j == KP - 1), perf_mode=DR,
                            )
                        # descale + bias here so gelu can batch bias-free
                        nc.vector.tensor_scalar(
                            out=a_raw[:, hc, :], in0=pf1, scalar1=DF1,
                            scalar2=f1b_sb[:, hc:hc + 1], op0=ALU.mult, op1=ALU.add,
                        )
                    fs.append(fc1_hc)
                return fs

            def gelu_fc2(c):
                for g in range(3):
                    nc.scalar.activation(
                        out=a_fm[:, 8 * g:8 * g + 8, :],
                        in_=a_raw[:, 8 * g:8 * g + 8, :],
                        func=AF.Gelu,
                    )
                for tl in range(4):
                    t = 4 * c + tl
                    facc = psM.tile([P, 512], F32, tag="mlp", name=f"facc_{t}")
                    facc2 = psM.tile([P, 512], F32, tag="mlp", name=f"facc2_{t}")
                    for jp in range(KP2):
                        nc.tensor.matmul(
                            facc,
                            a_fm[:, 2 * jp:2 * jp + 2, tl * P:(tl + 1) * P],
                            f2w_sb[:, jp, :, 0:512],
                            start=(jp == 0), stop=(jp == KP2 - 1), perf_mode=DR,
                        )
                    for jp in range(KP2):
                        nc.tensor.matmul(
                            facc2[:, 0:256],
                            a_fm[:, 2 * jp:2 * jp + 2, tl * P:(tl + 1) * P],
                            f2w_sb[:, jp, :, 512:768],
                            start=(jp == 0), stop=(jp == KP2 - 1), perf_mode=DR,
                        )
                    nc.vector.scalar_tensor_tensor(
                        out=x1[:, t, 0:512], in0=facc, scalar=DF2,
                        in1=x1[:, t, 0:512], op0=ALU.mult, op1=ALU.add,
                    )
                    nc.vector.scalar_tensor_tensor(
                        out=x1[:, t, 512:768], in0=facc2[:, 0:256], scalar=DF2,
                        in1=x1[:, t, 512:768], op0=ALU.mult, op1=ALU.add,
                    )
                    nc.sync.dma_start(out=out_r[:, t, :], in_=x1[:, t, :])

            # ---- main schedule ----
            emit_qk(0)
            attn_chunk(0, [])
            attn_chunk(1, proj_ln2_tr(0) + fc1_units(0))
            for f in proj_ln2_tr(1):
                f()
            gelu_fc2(0)
            for f in fc1_units(1):
                f()
            gelu_fc2(1)

            if "__dbg" in outs:
                dbg = outs["__dbg"]
                nc.sync.dma_start(out=dbg["hT"], in_=hT)
                nc.sync.dma_start(out=dbg["qkT"], in_=qkT)
                nc.sync.dma_start(out=dbg["vext"], in_=vext)
                nc.sync.dma_start(out=dbg["oT"], in_=oT)
                nc.sync.dma_start(out=dbg["h2T"], in_=h2T)
                nc.sync.dma_start(out=dbg["a_raw"], in_=a_raw)
                nc.sync.dma_start(out=dbg["a_fm"], in_=a_fm)


def fold_inputs(inputs):
    """Fold LN gamma/beta and v-bias into downstream weights; quantize the
    big weights to fp8-e4m3 in DoubleRow-interleaved layout (exact pow2
    scaling, descale applied in-kernel)."""
    import ml_dtypes

    E4 = ml_dtypes.float8_e4m3

    f = {k: np.asarray(v, dtype=np.float32) for k, v in inputs.items()}
    qkw = f["ln1_g"][:, None] * f["qk_w"]
    qkb = f["ln1_b"] @ f["qk_w"]
    vw = f["ln1_g"][:, None] * f["v_w"]
    vb = f["ln1_b"] @ f["v_w"]
    # softmax rows sum to 1 => o = attn @ (v + 1 vb^T) = attn@v + vb
    pb = f["proj_b"] + vb @ f["proj_w"]
    f1w = f["ln2_g"][:, None] * f["fc1_w"]
    f1b = f["fc1_b"] + f["ln2_b"] @ f["fc1_w"]

    def q8(w, kexp):
        # [K, M] -> [128, K//256, 2, M]: element (p, j, s, m) = w[256j+128s+p, m]
        K, M = w.shape
        wq = np.clip(w * (2.0 ** kexp), -240, 240).astype(E4)
        return np.ascontiguousarray(
            wq.reshape(K // 256, 2, P, M).transpose(2, 0, 1, 3)
        )

    return {
        "qkw": q8(qkw, WEXP),
        "qkb": np.ascontiguousarray(qkb),
        "vw": q8(vw, WEXP),
        "pw": q8(f["proj_w"], WEXP),
        "pbb": np.ascontiguousarray(np.broadcast_to(pb, (P, EMB)).copy()),
        "f1w": q8(f1w, WEXP),
        "f1b": np.ascontiguousarray(f1b),
        "f2w": q8(f["fc2_w"], WEXP2),
        "f2bb": np.ascontiguousarray(np.broadcast_to(f["fc2_b"], (P, EMB)).copy()),
    }


_INPUT_SHAPES = {
    "x": ((SEQ, EMB), F32),
    "qkw": ((P, KP, 2, 2 * EMB), FP8),
    "qkb": ((2 * EMB,), F32),
    "vw": ((P, KP, 2, EMB), FP8),
    "pw": ((P, KP, 2, EMB), FP8),
    "pbb": ((P, EMB), F32),
    "f1w": ((P, KP, 2, MLPD), FP8),
    "f1b": ((MLPD,), F32),
    "f2w": ((P, KP2, 2, EMB), FP8),
    "f2bb": ((P, EMB), F32),
}

_N_CORES = 8
_compiled = {}


def _build_nc(num_devices=_N_CORES):
    import concourse.tile as tile
    from concourse import bacc

    nc = bacc.Bacc(
        "TRN2", target_bir_lowering=False, debug=False, num_devices=num_devices
    )
    ins = {
        name: nc.dram_tensor(name, list(shape), dt, kind="ExternalInput").ap()
        for name, (shape, dt) in _INPUT_SHAPES.items()
    }
    out = nc.dram_tensor("out", [SEQ, EMB], F32, kind="ExternalOutput").ap()
    with tile.TileContext(nc) as tc:
        build_block(tc, {"out": out}, ins)
    nc.compile()
    return nc


def kernel(**inputs):
    """Full-input entry point: x [8, 1024, 768] + weights -> [8, 1024, 768]."""
    from concourse.bass_utils import run_bass_kernel_spmd

    if "nc" not in _compiled:
        _compiled["nc"] = _build_nc()
    nc = _compiled["nc"]

    x = np.asarray(inputs["x"], dtype=np.float32)
    folded = fold_inputs({k: v for k, v in inputs.items() if k != "x"})
    in_maps = [
        {"x": np.ascontiguousarray(x[c]), **folded} for c in range(_N_CORES)
    ]
    res = run_bass_kernel_spmd(nc, in_maps, core_ids=list(range(_N_CORES)))
    return np.stack([res.results[c]["out"] for c in range(_N_CORES)]).astype(
        np.float32
    )


# revision 12
# speedup vs baseline: 1.2586x; 1.0627x over previous
"""Transformer block kernel for TRN2 (Bass/Tile), one batch element per core.

fp8-e4m3 DoubleRow edition. All projections (qk, v, proj, fc1, fc2) run as
fp8 DoubleRow matmuls (256-deep contraction per pass, 2x fp8 throughput);
attention scores run fp8 normal-mode with two heads row-packed onto the PE
array (tile_position via base_partition 0/64); AV runs fp8 DoubleRow over
key-tile pairs with the softmax denominator riding as a ones column.

Numerics: per-tensor pow2 scales keep everything in e4m3 range
(weights x2^12/13, LN outputs x32 folded into rstd, attention o x256 folded
into the normalize, exp biased by ln8). Descales fold into existing DVE
tensor_scalar / scalar_tensor_tensor ops and ACT's input affine, so they
cost nothing. End-to-end L2 err ~1e-2 (budget 2e-2), CPU-simulated.

ACT table sets: LN rstd uses Ln+Exp (same set as attention's exp), and all
gelu is deferred to one block per chunk, so the ACT tables load only twice.

Pipeline (per core): LN1+transpose -> v -> qk(hp0) -> attn chunk c0
(queries 0..511, qk prefetch as PE filler) -> attn c1 with proj/LN2/
transpose/fc1 of c0 interleaved as PE fillers under the ACT-bound exp
stream -> tail: proj/LN2(c1), gelu+fc2(c0), fc1(c1), gelu+fc2(c1).
"""

import math
import sys
from contextlib import ExitStack

if "/opt/trn_rl_repo" not in sys.path:
    sys.path.insert(0, "/opt/trn_rl_repo")

import numpy as np

import concourse.bass as bass
import concourse.mybir as mybir
from concourse.masks import make_identity

F32 = mybir.dt.float32
BF16 = mybir.dt.bfloat16
FP8 = mybir.dt.float8e4
AF = mybir.ActivationFunctionType
ALU = mybir.AluOpType
DR = mybir.MatmulPerfMode.DoubleRow

P = 128
EMB = 768
SEQ = 1024
NH = 12
HD = 64
MLPD = 3072
EC = EMB // P       # 6 embedding chunks
NT = SEQ // P       # 8 token tiles
HC = MLPD // P      # 24 hidden chunks
HP = NH // 2        # 6 head pairs
KP = EMB // 256     # 3 k-pairs (DoubleRow) for 768 contraction
KP2 = MLPD // 256   # 12 k-pairs for 3072 contraction
EPS = 1e-5
SCALE = HD ** -0.5  # applied inside exp's input affine

# pow2 scale exponents (weights scaled on host, activations on device)
WEXP = 12           # qkw/vw/pw/f1w weight scale 2^12 (|w|max ~0.036 -> ~148)
WEXP2 = 13          # f2w scale 2^13 (|w|max ~0.018 -> ~148)
HEXP = 5            # LN outputs scaled x32 (folded into rstd)
OEXP = 8            # attention o scaled x256 (folded into normalize)
PBIAS = math.log(8.0)  # exp(s/8 + ln8): p in (0, ~128], cancels in softmax
DQK = 2.0 ** -(WEXP + HEXP)
DV = 2.0 ** -(WEXP + HEXP)
DPR = 2.0 ** -(WEXP + OEXP)
DF1 = 2.0 ** -(WEXP + HEXP)
DF2 = 2.0 ** -WEXP2


def _ln_stats(nc, x_ap, mv, stats, eps_t):
    """bn stats; mv[:,0]=mean, mv[:,1]=32/sqrt(var+eps) via Ln+Exp (one
    ACT table set with attention's exp; no Sqrt-set load)."""
    xg = x_ap.rearrange("p (g d) -> p g d", d=256)
    for g in range(3):
        nc.vector.bn_stats(out=stats[:, g, :], in_=xg[:, g, :])
    nc.vector.bn_aggr(out=mv, in_=stats)
    # ln((var+eps)/1024) then exp(-0.5 * that) = 32/sqrt(var+eps)
    nc.scalar.activation(
        out=mv[:, 1:2], in_=mv[:, 1:2], func=AF.Ln,
        bias=eps_t, scale=1.0 / 1024.0,
    )
    nc.scalar.activation(out=mv[:, 1:2], in_=mv[:, 1:2], func=AF.Exp, scale=-0.5)


def _ln_apply(nc, x_ap, h_out, mv):
    nc.vector.tensor_scalar(
        out=h_out,
        in0=x_ap,
        scalar1=mv[:, 0:1],
        scalar2=mv[:, 1:2],
        op0=ALU.subtract,
        op1=ALU.mult,
    )


def _transpose_fm(nc, pool, tag, src_tok, dstT, t, ident, copy_engine):
    """PE-transpose token-major src [128, EMB] into dstT [:, e, t*128:...]."""
    for g0, gn in ((0, 4), (4, 2)):
        ptr = pool.tile([P, 4, P], BF16, tag=tag, name=f"ptr_{dstT.tensor.name}_t{t}_{g0}")
        for jj in range(gn):
            nc.tensor.transpose(ptr[:, jj, :], src_tok[:, (g0 + jj) * P:(g0 + jj + 1) * P], ident)
        copy_engine(out=dstT[:, g0:g0 + gn, t * P:(t + 1) * P], in_=ptr[:, 0:gn, :])


def build_block(tc, outs, ins):
    nc = tc.nc
    x_d = ins["x"]
    out_d = outs["out"]
    out_r = out_d.rearrange("(t p) e -> p t e", p=P)

    with ExitStack() as ctx:
        consts = ctx.enter_context(tc.tile_pool(name="consts", bufs=1))
        ident = consts.tile([P, P], BF16)
        make_identity(nc, ident)
        qkb_sb = consts.tile([P, 2 * EC], F32)
        f1b_sb = consts.tile([P, HC], F32)
        eps_t = consts.tile([P, 1], F32)
        nc.vector.memset(eps_t, EPS / 1024.0)
        pbias_t = consts.tile([P, 1], F32)
        nc.vector.memset(pbias_t, PBIAS)

        glob = ctx.enter_context(tc.tile_pool(name="glob", bufs=1))
        x1 = glob.tile([P, NT, EMB], F32)          # residual stream, token-major
        hT = glob.tile([P, EC, SEQ], FP8)          # LN1 out x32, feature-major
        h2T = glob.tile([P, EC, SEQ], FP8)         # LN2 out x32, feature-major
        qkT = glob.tile([P, 2 * EC, SEQ], FP8)     # q (m 0..5) / k (m 6..11)
        vext = glob.tile([P, NT, NH, 68], FP8)     # v + ones col (64) + pad
        oT = glob.tile([P, EC, SEQ], FP8)          # attention out x256
        a_raw = glob.tile([P, HC, 512], BF16)      # pre-gelu (descaled+bias)
        a_fm = glob.tile([P, HC, 512], FP8)        # post-gelu
        qkw_sb = glob.tile([P, KP, 2, 2 * EMB], FP8)
        vw_sb = glob.tile([P, KP, 2, EMB], FP8)
        pw_sb = glob.tile([P, KP, 2, EMB], FP8)
        f1w_sb = glob.tile([P, KP, 2, MLPD], FP8)
        f2w_sb = glob.tile([P, KP2, 2, EMB], FP8)
        pbb_sb = glob.tile([P, EMB], F32)
        f2bb_sb = glob.tile([P, EMB], F32)

        work = ctx.enter_context(tc.tile_pool(name="work", bufs=3))
        stat = ctx.enter_context(tc.tile_pool(name="stat", bufs=4))

        # ---- load x, LN1, transpose to hT; then x1 += proj-bias ----
        x_r = x_d.rearrange("(t p) e -> p t e", p=P)
        for t in range(NT):
            nc.sync.dma_start(out=x1[:, t, :], in_=x_r[:, t, :])

        with tc.tile_pool(name="psA", space="PSUM", bufs=2) as psA:
            mvs = []
            for t in range(NT):
                mv = stat.tile([P, 2], F32, tag="mv", bufs=NT, name=f"mv1_{t}")
                stats = stat.tile([P, 3, 6], F32, tag="stats", name=f"st1_{t}")
                _ln_stats(nc, x1[:, t, :], mv, stats, eps_t)
                mvs.append(mv)
            for t in range(NT):
                h_t = work.tile([P, EMB], BF16, tag="h", bufs=4, name=f"h1_{t}")
                _ln_apply(nc, x1[:, t, :], h_t, mvs[t])
                _transpose_fm(nc, psA, "trA", h_t, hT, t, ident, nc.scalar.copy)

        # weights / biases (emitted after x+LN so the x DMAs win the queues)
        nc.sync.dma_start(out=qkw_sb, in_=ins["qkw"])
        nc.sync.dma_start(out=vw_sb, in_=ins["vw"])
        nc.sync.dma_start(out=qkb_sb, in_=ins["qkb"].rearrange("(m p) -> p m", p=P))
        nc.sync.dma_start(out=pbb_sb, in_=ins["pbb"])
        nc.sync.dma_start(out=pw_sb, in_=ins["pw"])
        nc.sync.dma_start(out=f1w_sb, in_=ins["f1w"])
        nc.sync.dma_start(out=f1b_sb, in_=ins["f1b"].rearrange("(m p) -> p m", p=P))
        nc.sync.dma_start(out=f2w_sb, in_=ins["f2w"])
        nc.sync.dma_start(out=f2bb_sb, in_=ins["f2bb"])

        for t in range(NT):
            nc.vector.tensor_tensor(
                out=x1[:, t, :], in0=x1[:, t, :], in1=pbb_sb, op=ALU.add
            )

        with tc.tile_pool(name="psS", space="PSUM", bufs=2) as psS, \
             tc.tile_pool(name="psO", space="PSUM", bufs=2) as psO, \
             tc.tile_pool(name="psM", space="PSUM", bufs=2) as psM:

            # ---- v projection (flipped: stationary=hT, moving=vw) ----
            nc.vector.memset(vext[:, :, :, 64:65], 1.0)
            nc.vector.memset(vext[:, :, :, 65:68], 0.0)
            for t in range(NT):
                pv = psM.tile([P, 512], F32, tag="mlp", name=f"pv_{t}")
                pv2 = psM.tile([P, 512], F32, tag="mlp", name=f"pv2_{t}")
                for j in range(KP):
                    nc.tensor.matmul(
                        pv,
                        hT[:, 2 * j:2 * j + 2, t * P:(t + 1) * P],
                        vw_sb[:, j, :, 0:512],
                        start=(j == 0), stop=(j == KP - 1), perf_mode=DR,
                    )
                for j in range(KP):
                    nc.tensor.matmul(
                        pv2[:, 0:256],
                        hT[:, 2 * j:2 * j + 2, t * P:(t + 1) * P],
                        vw_sb[:, j, :, 512:768],
                        start=(j == 0), stop=(j == KP - 1), perf_mode=DR,
                    )
                nc.vector.tensor_scalar_mul(
                    out=vext[:, t, 0:8, 0:64],
                    in0=pv.rearrange("p (h d) -> p h d", d=HD),
                    scalar1=DV,
                )
                nc.vector.tensor_scalar_mul(
                    out=vext[:, t, 8:12, 0:64],
                    in0=pv2[:, 0:256].rearrange("p (h d) -> p h d", d=HD),
                    scalar1=DV,
                )

            # ---- qk projection for one head pair (2 m-tiles) ----
            def emit_qk(hp):
                for m in (hp, HP + hp):
                    for n2 in range(2):
                        pqk = psM.tile([P, 512], F32, tag="mlp", name=f"pqk_{m}_{n2}")
                        for j in range(KP):
                            nc.tensor.matmul(
                                pqk,
                                qkw_sb[:, j, :, m * P:(m + 1) * P],
                                hT[:, 2 * j:2 * j + 2, n2 * 512:(n2 + 1) * 512],
                                start=(j == 0), stop=(j == KP - 1), perf_mode=DR,
                            )
                        nc.vector.tensor_scalar(
                            out=qkT[:, m, n2 * 512:(n2 + 1) * 512],
                            in0=pqk,
                            scalar1=DQK,
                            scalar2=qkb_sb[:, m:m + 1],
                            op0=ALU.mult,
                            op1=ALU.add,
                        )

            def emit_some(fillers, fi, budget):
                n = 0
                while fi < len(fillers) and n < budget:
                    fillers[fi]()
                    fi += 1
                    n += 1
                return fi

            # ---- attention for one 512-query chunk ----
            def attn_chunk(c, fillers):
                fi = 0
                dpack = stat.tile([NH, 512], F32, tag="dpack", bufs=1,
                                  name=f"dp_c{c}")
                ous_all = [None] * NH
                for hp in range(HP):
                    if c == 0 and hp + 1 < HP:
                        emit_qk(hp + 1)
                    po = [
                        psO.tile([68, 512], F32, tag="po", name=f"po_c{c}h{2 * hp + s}")
                        for s in range(2)
                    ]
                    for i in range(4):  # key-tile pairs
                        pst = [
                            psS.tile([P, 2, 512], F32, tag="ps",
                                     name=f"ps_c{c}hp{hp}i{i}s{s}")
                            for s in range(2)
                        ]
                        # interleave heads A/B: adjacent MMs hit different
                        # PE row strips (base_partition 0 / 64) and overlap
                        for kk in range(2):
                            kt = 2 * i + kk
                            for sub in range(2):
                                doff = sub * HD
                                nc.tensor.matmul(
                                    pst[sub][:, kk, :],
                                    qkT[doff:doff + HD, HP + hp, kt * P:(kt + 1) * P],
                                    qkT[doff:doff + HD, hp, c * 512:(c + 1) * 512],
                                    start=True, stop=True,
                                )
                        pps = []
                        for sub in range(2):
                            ppt = work.tile([P, 2, 512], FP8, tag="pp", bufs=3,
                                            name=f"pp_c{c}hp{hp}i{i}s{sub}")
                            nc.scalar.activation(
                                out=ppt, in_=pst[sub], func=AF.Exp,
                                bias=pbias_t, scale=SCALE,
                            )
                            pps.append(ppt)
                        fi = emit_some(fillers, fi, 1)
                        for sub in range(2):
                            h = 2 * hp + sub
                            nc.tensor.matmul(
                                po[sub],
                                vext[:, 2 * i:2 * i + 2, h, :],
                                pps[sub],
                                start=(i == 0), stop=(i == 3), perf_mode=DR,
                            )
                    # stage numerators + denominators (one batched reciprocal
                    # per chunk; normalize happens after the hp loop)
                    for sub in range(2):
                        ou = work.tile([65, 512], F32, tag="ou", bufs=NH,
                                       name=f"ou_c{c}hp{hp}s{sub}")
                        nc.vector.tensor_copy(out=ou, in_=po[sub][0:65, :])
                        nc.sync.dma_start(
                            out=dpack[2 * hp + sub:2 * hp + sub + 1, :],
                            in_=ou[64:65, :],
                        )
                        ous_all[2 * hp + sub] = ou
                    fi = emit_some(fillers, fi, 1)
                rpack = stat.tile([NH, 512], F32, tag="rpack", bufs=1,
                                  name=f"rp_c{c}")
                nc.vector.reciprocal(out=rpack, in_=dpack)
                for h in range(NH):
                    hp, sub = h // 2, h % 2
                    rtmp = stat.tile([1, 512], F32, tag="rtmp", bufs=2,
                                     name=f"rt_c{c}h{h}")
                    nc.sync.dma_start(out=rtmp, in_=rpack[h:h + 1, :])
                    rb = work.tile([HD, 512], F32, tag="rb", bufs=3,
                                   name=f"rb_c{c}h{h}")
                    nc.gpsimd.partition_broadcast(rb, rtmp)
                    nc.vector.scalar_tensor_tensor(
                        out=oT[sub * HD:sub * HD + HD, hp, c * 512:(c + 1) * 512],
                        in0=ous_all[h][0:HD, :],
                        scalar=float(2 ** OEXP),
                        in1=rb,
                        op0=ALU.mult, op1=ALU.mult,
                    )
                emit_some(fillers, fi, len(fillers))

            # ---- MLP pieces for chunk c (emitted as fillers / tail) ----
            def proj_ln2_tr(c):
                fs = []
                for tl in range(4):
                    t = 4 * c + tl

                    def proj_t(t=t):
                        ppr = psM.tile([P, 512], F32, tag="mlp", name=f"ppr_{t}")
                        ppr2 = psM.tile([P, 512], F32, tag="mlp", name=f"ppr2_{t}")
                        for j in range(KP):
                            nc.tensor.matmul(
                                ppr,
                                oT[:, 2 * j:2 * j + 2, t * P:(t + 1) * P],
                                pw_sb[:, j, :, 0:512],
                                start=(j == 0), stop=(j == KP - 1), perf_mode=DR,
                            )
                        for j in range(KP):
                            nc.tensor.matmul(
                                ppr2[:, 0:256],
                                oT[:, 2 * j:2 * j + 2, t * P:(t + 1) * P],
                                pw_sb[:, j, :, 512:768],
                                start=(j == 0), stop=(j == KP - 1), perf_mode=DR,
                            )
                        nc.vector.scalar_tensor_tensor(
                            out=x1[:, t, 0:512], in0=ppr, scalar=DPR,
                            in1=x1[:, t, 0:512], op0=ALU.mult, op1=ALU.add,
                        )
                        nc.vector.scalar_tensor_tensor(
                            out=x1[:, t, 512:768], in0=ppr2[:, 0:256], scalar=DPR,
                            in1=x1[:, t, 512:768], op0=ALU.mult, op1=ALU.add,
                        )
                        if "__dbg" in outs:
                            nc.sync.dma_start(
                                out=outs["__dbg"]["x1p"].rearrange(
                                    "(t p) e -> p t e", p=P)[:, t, :],
                                in_=x1[:, t, :],
                            )

                    def ln2_t(t=t):
                        mv = stat.tile([P, 2], F32, tag="mv", bufs=NT, name=f"mv2_{t}")
                        stats = stat.tile([P, 3, 6], F32, tag="stats", name=f"st2_{t}")
                        _ln_stats(nc, x1[:, t, :], mv, stats, eps_t)
                        h_t = work.tile([P, EMB], BF16, tag="h", bufs=4, name=f"h2_{t}")
                        _ln_apply(nc, x1[:, t, :], h_t, mv)
                        # f2 bias joins the stream only after LN2 consumed x1
                        nc.vector.tensor_tensor(
                            out=x1[:, t, :], in0=x1[:, t, :], in1=f2bb_sb, op=ALU.add
                        )
                        _transpose_fm(nc, psS, "ps", h_t, h2T, t, ident,
                                      nc.vector.tensor_copy)

                    fs.append(proj_t)
                    fs.append(ln2_t)
                return fs

            def fc1_units(c):
                fs = []
                for hc in range(HC):
                    def fc1_hc(hc=hc, c=c):
                        pf1 = psM.tile([P, 512], F32, tag="mlp", name=f"pf1_c{c}_{hc}")
                        for j in range(KP):
                            nc.tensor.matmul(
                                pf1,
                                f1w_sb[:, j, :, hc * P:(hc + 1) * P],
                                h2T[:, 2 * j:2 * j + 2, c * 512:(c + 1) * 512],
                                start=(j == 0), stop=(j == KP - 1), perf_mode=DR,
                            )
                        # descale + bias here so gelu can batch bias-free
                        nc.vector.tensor_scalar(
                            out=a_raw[:, hc, :], in0=pf1, scalar1=DF1,
                            scalar2=f1b_sb[:, hc:hc + 1], op0=ALU.mult, op1=ALU.add,
                        )
                    fs.append(fc1_hc)
                return fs

            def gelu_fc2(c):
                for g in range(3):
                    nc.scalar.activation(
                        out=a_fm[:, 8 * g:8 * g + 8, :],
                        in_=a_raw[:, 8 * g:8 * g + 8, :],
                        func=AF.Gelu,
                    )
                for tl in range(4):
                    t = 4 * c + tl
                    facc = psM.tile([P, 512], F32, tag="mlp", name=f"facc_{t}")
                    facc2 = psM.tile([P, 512], F32, tag="mlp", name=f"facc2_{t}")
                    for jp in range(KP2):
                        nc.tensor.matmul(
                            facc,
                            a_fm[:, 2 * jp:2 * jp + 2, tl * P:(tl + 1) * P],
                            f2w_sb[:, jp, :, 0:512],
                            start=(jp == 0), stop=(jp == KP2 - 1), perf_mode=DR,
                        )
                    for jp in range(KP2):
                        nc.tensor.matmul(
                            facc2[:, 0:256],
                            a_fm[:, 2 * jp:2 * jp + 2, tl * P:(tl + 1) * P],
                            f2w_sb[:, jp, :, 512:768],
                            start=(jp == 0), stop=(jp == KP2 - 1), perf_mode=DR,
                        )
                    nc.vector.scalar_tensor_tensor(
                        out=x1[:, t, 0:512], in0=facc, scalar=DF2,
                        in1=x1[:, t, 0:512], op0=ALU.mult, op1=ALU.add,
                    )
                    nc.vector.scalar_tensor_tensor(
                        out=x1[:, t, 512:768], in0=facc2[:, 0:256], scalar=DF2,
                        in1=x1[:, t, 512:768], op0=ALU.mult, op1=ALU.add,
                    )
                    nc.sync.dma_start(out=out_r[:, t, :], in_=x1[:, t, :])

            # ---- main schedule ----
            emit_qk(0)
            attn_chunk(0, [])
            attn_chunk(1, proj_ln2_tr(0) + fc1_units(0))
            for f in proj_ln2_tr(1):
                f()
            gelu_fc2(0)
            for f in fc1_units(1):
                f()
            gelu_fc2(1)

            if "__dbg" in outs:
                dbg = outs["__dbg"]
                nc.sync.dma_start(out=dbg["hT"], in_=hT)
                nc.sync.dma_start(out=dbg["qkT"], in_=qkT)
                nc.sync.dma_start(out=dbg["vext"], in_=vext)
                nc.sync.dma_start(out=dbg["oT"], in_=oT)
                nc.sync.dma_start(out=dbg["h2T"], in_=h2T)
                nc.sync.dma_start(out=dbg["a_raw"], in_=a_raw)
                nc.sync.dma_start(out=dbg["a_fm"], in_=a_fm)


def fold_inputs(inputs):
    """Fold LN gamma/beta and v-bias into downstream weights; quantize the
    big weights to fp8-e4m3 in DoubleRow-interleaved layout (exact pow2
    scaling, descale applied in-kernel)."""
    import ml_dtypes

    E4 = ml_dtypes.float8_e4m3

    f = {k: np.asarray(v, dtype=np.float32) for k, v in inputs.items()}
    qkw = f["ln1_g"][:, None] * f["qk_w"]
    qkb = f["ln1_b"] @ f["qk_w"]
    vw = f["ln1_g"][:, None] * f["v_w"]
    vb = f["ln1_b"] @ f["v_w"]
    # softmax rows sum to 1 => o = attn @ (v + 1 vb^T) = attn@v + vb
    pb = f["proj_b"] + vb @ f["proj_w"]
    f1w = f["ln2_g"][:, None] * f["fc1_w"]
    f1b = f["fc1_b"] + f["ln2_b"] @ f["fc1_w"]

    def q8(w, kexp):
        # [K, M] -> [128, K//256, 2, M]: element (p, j, s, m) = w[256j+128s+p, m]
        K, M = w.shape
        wq = np.clip(w * (2.0 ** kexp), -240, 240).astype(E4)
        return np.ascontiguousarray(
            wq.reshape(K // 256, 2, P, M).transpose(2, 0, 1, 3)
        )

    return {
        "qkw": q8(qkw, WEXP),
        "qkb": np.ascontiguousarray(qkb),
        "vw": q8(vw, WEXP),
        "pw": q8(f["proj_w"], WEXP),
        "pbb": np.ascontiguousarray(np.broadcast_to(pb, (P, EMB)).copy()),
        "f1w": q8(f1w, WEXP),
        "f1b": np.ascontiguousarray(f1b),
        "f2w": q8(f["fc2_w"], WEXP2),
        "f2bb": np.ascontiguousarray(np.broadcast_to(f["fc2_b"], (P, EMB)).copy()),
    }


_INPUT_SHAPES = {
    "x": ((SEQ, EMB), F32),
    "qkw": ((P, KP, 2, 2 * EMB), FP8),
    "qkb": ((2 * EMB,), F32),
    "vw": ((P, KP, 2, EMB), FP8),
    "pw": ((P, KP, 2, EMB), FP8),
    "pbb": ((P, EMB), F32),
    "f1w": ((P, KP, 2, MLPD), FP8),
    "f1b": ((MLPD,), F32),
    "f2w": ((P, KP2, 2, EMB), FP8),
    "f2bb": ((P, EMB), F32),
}

_N_CORES = 8
_compiled = {}


def _build_nc(num_devices=_N_CORES):
    import concourse.tile as tile
    from concourse import bacc

    # Force exp and ln onto the shared natural_log_exp_and_others ACT table
    # set: empty the competing sets (indices preserved) so the table-load
    # pass cannot alternate between per-function sets (~2.7us per reload).
    if not getattr(bacc, "_act_tables_patched", False):
        _orig_gat = bacc.get_activation_tables

        def _gat(arch):
            tabs = _orig_gat(arch)
            if "natural_log_exp_and_others" in tabs:
                for kill in ("exp_and_others", "natural_log", "exp_and_friends"):
                    if kill in tabs:
                        tabs[kill] = set()
            return tabs

        bacc.get_activation_tables = _gat
        bacc._act_tables_patched = True

    nc = bacc.Bacc(
        "TRN2", target_bir_lowering=False, debug=False, num_devices=num_devices
    )
    ins = {
        name: nc.dram_tensor(name, list(shape), dt, kind="ExternalInput").ap()
        for name, (shape, dt) in _INPUT_SHAPES.items()
    }
    out = nc.dram_tensor("out", [SEQ, EMB], F32, kind="ExternalOutput").ap()
    with tile.TileContext(nc) as tc:
        build_block(tc, {"out": out}, ins)
    nc.compile()
    return nc


def kernel(**inputs):
    """Full-input entry point: x [8, 1024, 768] + weights -> [8, 1024, 768]."""
    from concourse.bass_utils import run_bass_kernel_spmd

    if "nc" not in _compiled:
        _compiled["nc"] = _build_nc()
    nc = _compiled["nc"]

    x = np.asarray(inputs["x"], dtype=np.float32)
    folded = fold_inputs({k: v for k, v in inputs.items() if k != "x"})
    in_maps = [
        {"x": np.ascontiguousarray(x[c]), **folded} for c in range(_N_CORES)
    ]
    res = run_bass_kernel_spmd(nc, in_maps, core_ids=list(range(_N_CORES)))
    return np.stack([res.results[c]["out"] for c in range(_N_CORES)]).astype(
        np.float32
    )
